# revision 4
# baseline (speedup 1.0000x reference)
"""Gemma3 sliding-window attention layer on 8 Trainium2 NeuronCores. (v2)

Tensor-parallel over query heads: core h computes query head h (kv head
h//2), i.e. column-parallel qkv projection, full per-head attention, and
the row-parallel slice of o_proj; the 8 partial [S, H] outputs are summed
on the host (the all-reduce / unshard step).

v2 restructure vs v1: the four 512-column sequence groups are processed
as a software pipeline — for each group: k-proj, q-proj (PE) with the
RMSNorm/RoPE (ACT/DVE) of the same group overlapped under the v-proj and
the next group's projection matmuls, then attention + o_proj for the
group's 4 row blocks immediately after.  The PE therefore never idles
between "phases", which also keeps the HAM clock-gate warm.

Other changes: the (1+w) RMSNorm gains are folded into the projection
weights (and 1/(1+w)^2 into the column-sum matmul lhsT), so the RoPE
tables are just cos/sin halves shared by q and k (2MB instead of 8MB of
DMA); output partials are stored bf16 (host sums in f32); DMAs are
batched into fewer, larger transfers.
"""

import numpy as np
import ml_dtypes

import concourse.bass as bass
import concourse.mybir as mybir
import concourse.tile as tile
from concourse.bass_utils import run_bass_kernel_spmd
from concourse.masks import make_identity

# ---- problem constants (hardcoded; kernel.py must be self-contained) ----
S = 2048          # sequence length
H = 2560          # hidden size
NH = 8            # query heads
NKV = 4           # kv heads
D = 256           # head dim
EPS = 1e-6
SCALING = 256.0 ** -0.5
WINDOW = 1024 - 1  # sliding window - 1

N_CORES = 8
KC = H // 128      # 20 contraction chunks for the projection
RB = S // 128      # 16 row blocks
MASK_VAL = -1e10

F32 = mybir.dt.float32
F32R = mybir.dt.float32r
BF16 = mybir.dt.bfloat16
BLK_WIN = WINDOW // 128 + 1   # 8: c in [r-8, r] can contribute
GW = 512                      # sequence-column group width
NG = S // GW                  # 4 groups
RPG = GW // 128               # 4 row blocks per group


def _bf16(x):
    return np.ascontiguousarray(x.astype(ml_dtypes.bfloat16))


def _chunk_part(x, p=128):
    """[c*p, n] -> [p, c, n] host relayout so it DMAs 1:1 into an SBUF tile."""
    c = x.shape[0] // p
    return np.ascontiguousarray(
        x.reshape(c, p, *x.shape[1:]).transpose(1, 0, *range(2, x.ndim + 1))
    )


def split_multiwaits(nc):
    """This toolchain's codegen allows one sync-wait slot per instruction.

    Tile emits several waits on the first consumer of multi-queue DMAs and
    on kernel-tail drains; hoist all but the last wait onto same-engine
    NoOps inserted immediately before the offending instruction (queue
    order on the engine preserves the semantics exactly).
    """
    k = 0
    for f in nc.m.functions:
        for bb in f.blocks:
            insts = bb.instructions
            if not any(i.sync_info and len(i.sync_info.on_wait) > 1
                       for i in insts):
                continue
            newlist = []
            for inst in insts:
                si = inst.sync_info
                if si is not None and len(si.on_wait) > 1:
                    for w in list(si.on_wait)[:-1]:
                        nop = mybir.InstNoOp(name=f"{inst.name}-ws{k}")
                        k += 1
                        nop.engine = inst.engine
                        nop.sync_info = mybir.SyncInfo(on_wait=[w], on_update=[])
                        newlist.append(nop)
                    inst.sync_info = mybir.SyncInfo(
                        on_wait=[list(si.on_wait)[-1]],
                        on_update=list(si.on_update))
                newlist.append(inst)
            live = bb.instructions
            live.clear()
            live.extend(newlist)
    return nc


def build_nc():
    """One-core SPMD program (all cores run this; data differs per core)."""
    nc = bass.Bass()

    # inputs (per core unless noted; several are identical across cores)
    hT_d = nc.declare_dram_parameter("hT", [128, NG, KC, GW], BF16, isOutput=False)
    wq_d = nc.declare_dram_parameter("wq", [128, KC, D], BF16, isOutput=False)
    wk_d = nc.declare_dram_parameter("wk", [128, KC, D], BF16, isOutput=False)
    wv_d = nc.declare_dram_parameter("wv", [128, KC, D], BF16, isOutput=False)
    cos_d = nc.declare_dram_parameter("cosH", [128, S], BF16, isOutput=False)
    sin_d = nc.declare_dram_parameter("sinH", [128, S], BF16, isOutput=False)
    gq_d = nc.declare_dram_parameter("Gq", [128, 2, 128], F32R, isOutput=False)
    gk_d = nc.declare_dram_parameter("Gk", [128, 2, 128], F32R, isOutput=False)
    wo_d = nc.declare_dram_parameter("wo", [128, 2, H], BF16, isOutput=False)
    bm_d = nc.declare_dram_parameter("bmask", [128, 256], BF16, isOutput=False)
    out_d = nc.declare_dram_parameter("out", [S, H], BF16, isOutput=True)

    with tile.TileContext(nc) as tc:
        with (
            tc.tile_pool(name="persist", bufs=1) as persist,
            tc.tile_pool(name="ht", bufs=2) as htpool,
            tc.tile_pool(name="rawpool", bufs=2) as rawpool,
            tc.tile_pool(name="npool", bufs=2) as npool,
            tc.tile_pool(name="rstdpool", bufs=2) as rstdpool,
            tc.tile_pool(name="probs", bufs=8) as prpool,
            tc.tile_pool(name="asmall", bufs=4) as aspool,
            tc.tile_pool(name="opool", bufs=2) as opool,
            tc.tile_pool(name="ps512", bufs=5, space="PSUM") as ps512,
            tc.tile_pool(name="ps257", bufs=2, space="PSUM") as ps257,
            tc.tile_pool(name="ps128", bufs=1, space="PSUM") as ps128,
        ):
            # tensors that live across the whole kernel
            qTf = persist.tile([128, 2, S], BF16, tag="qTf")  # normed+roped
            kTf = persist.tile([128, 2, S], BF16, tag="kTf")
            v_aug = persist.tile([128, RB, D + 1], BF16, tag="vaug")
            attnT = persist.tile([128, 2, S], BF16, tag="attnT")
            cos_sb = persist.tile([128, S], BF16, tag="cos")
            sin_sb = persist.tile([128, S], BF16, tag="sin")
            gq_sb = persist.tile([128, 2, 128], F32R, tag="gq")
            gk_sb = persist.tile([128, 2, 128], F32R, tag="gk")
            bmask = persist.tile([128, 256], BF16, tag="bm")
            ident = persist.tile([128, 128], BF16, tag="ident")
            wq_sb = persist.tile([128, KC, D], BF16, tag="wq")
            wk_sb = persist.tile([128, KC, D], BF16, tag="wk")
            wv_sb = persist.tile([128, KC, D], BF16, tag="wv")
            wo_sb = persist.tile([128, 2, H], BF16, tag="wo")

            # critical-path DMAs first: wk + the first group's hT chunks
            # gate the first matmuls; everything else can trickle in later.
            # Both arrive in 4-kc chunks, interleaved, so the kc-loop of the
            # first projection starts after ~1/5 of the transfers.
            ht0 = []
            for dq in range(5):
                nc.sync.dma_start(out=wk_sb[:, dq * 4:(dq + 1) * 4, :],
                                  in_=wk_d[:, dq * 4:(dq + 1) * 4, :])
                t = htpool.tile([128, 4, GW], BF16, tag=f"ht{dq}")
                nc.sync.dma_start(out=t, in_=hT_d[:, 0, dq * 4:(dq + 1) * 4, :])
                ht0.append(t)
            nc.sync.dma_start(out=wq_sb, in_=wq_d[:])
            nc.sync.dma_start(out=wv_sb, in_=wv_d[:])
            nc.sync.dma_start(out=gk_sb, in_=gk_d[:])
            nc.sync.dma_start(out=cos_sb, in_=cos_d[:])
            nc.sync.dma_start(out=sin_sb, in_=sin_d[:])
            nc.sync.dma_start(out=gq_sb, in_=gq_d[:])
            nc.sync.dma_start(out=bmask, in_=bm_d[:])
            nc.sync.dma_start(out=wo_sb, in_=wo_d[:])
            make_identity(nc, ident)
            eps_q = persist.tile([128, 1], F32, tag="eps_q")
            eps_k = persist.tile([128, 1], F32, tag="eps_k")
            nc.vector.memset(eps_q, EPS / (SCALING * SCALING))
            nc.vector.memset(eps_k, EPS)

            def proj_qk(w_sb, raw, ht):
                """q/k projection for one group: [d, 512] transposed layout."""
                for dc in range(2):
                    ps = ps512.tile([128, GW], F32, tag="ps512")
                    for kc in range(KC):
                        nc.tensor.matmul(
                            ps,
                            lhsT=w_sb[:, kc, dc * 128:(dc + 1) * 128],
                            rhs=ht[kc // 4][:, kc % 4, :],
                            start=(kc == 0), stop=(kc == KC - 1),
                        )
                    if dc == 0:
                        nc.scalar.copy(raw[:, dc, :], ps)
                    else:
                        nc.vector.tensor_copy(raw[:, dc, :], ps)

            def norm_rope(raw, fin, g_sb, eps, s2, g):
                """RMSNorm stats + RoPE + bf16 cast for one 512-col group.

                fin[0] = (raw[0]*cos - raw[1]*sin) * rstd
                fin[1] = (raw[1]*cos + raw[0]*sin) * rstd
                rstd = 1/sqrt((sum_d raw_d^2 * gain_d)/(D*s2) + eps/s2),
                broadcast to all partitions by the G ones-style matmul.
                """
                sl = slice(g * GW, (g + 1) * GW)
                sq0 = npool.tile([128, GW], F32R, tag="sq0")
                sq1 = npool.tile([128, GW], F32R, tag="sq1")
                nc.scalar.square(sq0, raw[:, 0, :])
                nc.scalar.square(sq1, raw[:, 1, :])
                pss = ps512.tile([128, GW], F32, tag="ps512")
                for dc, sq in enumerate((sq0, sq1)):
                    nc.tensor.matmul(
                        pss, lhsT=g_sb[:, dc, :], rhs=sq,
                        start=(dc == 0), stop=(dc == 1),
                    )
                # rstd = (mean_sq + eps)^-1/2 computed as exp(-ln(x)/2): both
                # steps on ACT, so the DVE FIFO never head-blocks on it
                lnx = rstdpool.tile([128, GW], F32, tag="lnx")
                nc.scalar.activation(
                    out=lnx, in_=pss,
                    func=mybir.ActivationFunctionType.Ln,
                    scale=1.0 / (D * s2), bias=eps,
                )
                rstd = rstdpool.tile([128, GW], BF16, tag="rstd")
                nc.scalar.activation(
                    out=rstd, in_=lnx,
                    func=mybir.ActivationFunctionType.Exp,
                    scale=-0.5,
                )
                # rotate-half combines (rstd-independent) first, the two
                # rstd multiplies last
                t1 = npool.tile([128, GW], BF16, tag="t1")
                t2 = npool.tile([128, GW], BF16, tag="t2")
                # dc = 0: cos*raw0 - sin*raw1
                nc.vector.tensor_mul(t1, raw[:, 0, :], cos_sb[:, sl])
                nc.vector.tensor_mul(t2, raw[:, 1, :], sin_sb[:, sl])
                nc.vector.tensor_sub(t1, t1, t2)
                # dc = 1: cos*raw1 + sin*raw0
                t3 = npool.tile([128, GW], BF16, tag="t3")
                t4 = npool.tile([128, GW], BF16, tag="t4")
                nc.vector.tensor_mul(t3, raw[:, 1, :], cos_sb[:, sl])
                nc.vector.tensor_mul(t4, raw[:, 0, :], sin_sb[:, sl])
                nc.vector.tensor_add(t3, t3, t4)
                nc.vector.tensor_mul(fin[:, 0, sl], t1, rstd)
                nc.vector.tensor_mul(fin[:, 1, sl], t3, rstd)

            def attention_row(r):
                """scores -> softmax -> attn·v -> attnT for one row block.

                The in-window interior key blocks need no mask; the diagonal
                and (when the window is full) the leading partial block are
                zeroed AFTER the exp with a 0/1 bf16 mask multiply, and are
                ordered LAST so their extra DVE hop hides under the interior
                blocks' attn·v matmuls.
                """
                cmin = max(0, r - BLK_WIN)
                interior = [c for c in range(cmin, r + 1)
                            if c != r and c != r - BLK_WIN]
                specials = ([r - BLK_WIN] if r - BLK_WIN >= 0 else []) + [r]
                # pass 1: scores + exp for interior chunks (batch 4 col
                # blocks per PSUM bank so one Exp covers them)
                order = []   # (c, pT, j)
                for i0 in range(0, len(interior), 4):
                    chunk = interior[i0:i0 + 4]
                    w = len(chunk) * 128
                    psW = ps512.tile([128, GW], F32, tag="ps512")
                    for j, c in enumerate(chunk):
                        for dc in range(2):
                            nc.tensor.matmul(
                                psW[:, j * 128:(j + 1) * 128],
                                lhsT=kTf[:, dc, c * 128:(c + 1) * 128],
                                rhs=qTf[:, dc, r * 128:(r + 1) * 128],
                                start=(dc == 0), stop=(dc == 1),
                            )
                    pT = prpool.tile([128, GW], BF16, tag="pT")
                    nc.scalar.activation(
                        out=pT[:, :w], in_=psW[:, :w],
                        func=mybir.ActivationFunctionType.Exp,
                    )
                    order.extend((c, pT, j) for j, c in enumerate(chunk))
                # special chunk: exp then 0/1-mask multiply (DVE)
                wsp = len(specials) * 128
                psW = ps512.tile([128, GW], F32, tag="ps512")
                for j, c in enumerate(specials):
                    for dc in range(2):
                        nc.tensor.matmul(
                            psW[:, j * 128:(j + 1) * 128],
                            lhsT=kTf[:, dc, c * 128:(c + 1) * 128],
                            rhs=qTf[:, dc, r * 128:(r + 1) * 128],
                            start=(dc == 0), stop=(dc == 1),
                        )
                pT = prpool.tile([128, GW], BF16, tag="pT")
                nc.scalar.activation(
                    out=pT[:, :wsp], in_=psW[:, :wsp],
                    func=mybir.ActivationFunctionType.Exp,
                )
                moff = 256 - wsp   # [partial, diag] or just [diag]
                nc.vector.tensor_mul(
                    pT[:, :wsp], pT[:, :wsp], bmask[:, moff:256])
                order.extend((c, pT, j) for j, c in enumerate(specials))
                return order

            def attnv_row(order):
                """pass 2: attn·v accumulation (interiors first, specials
                last so their mask multiply hides under the matmuls)."""
                ps_at = ps257.tile([128, D + 1], F32, tag="ps257")
                for i, (c, pT, j) in enumerate(order):
                    nc.tensor.matmul(
                        ps_at,
                        lhsT=pT[:, j * 128:(j + 1) * 128],
                        rhs=v_aug[:, c, :],
                        start=(i == 0), stop=(i == len(order) - 1),
                    )
                rc = aspool.tile([128, 1], F32, tag="rc")
                nc.vector.reciprocal(rc, ps_at[:, D:D + 1])
                a_sb = aspool.tile([128, D], BF16, tag="asb")
                nc.scalar.mul(a_sb, ps_at[:, 0:D], rc)
                return a_sb

            def finish_row(r, a_sb):
                """transpose attn row into attnT (feeds o_proj as lhsT)."""
                for dc in range(2):
                    pt = ps128.tile([128, 128], BF16, tag="ps128")
                    nc.tensor.transpose(
                        pt, a_sb[:, dc * 128:(dc + 1) * 128], ident
                    )
                    nc.vector.tensor_copy(
                        attnT[:, dc, r * 128:(r + 1) * 128], pt
                    )

            def o_proj_row(r):
                o_sb = opool.tile([128, H], BF16, tag="osb")
                for hc in range(H // 512):
                    ps = ps512.tile([128, 512], F32, tag="ps512")
                    for dc in range(2):
                        nc.tensor.matmul(
                            ps,
                            lhsT=attnT[:, dc, r * 128:(r + 1) * 128],
                            rhs=wo_sb[:, dc, hc * 512:(hc + 1) * 512],
                            start=(dc == 0), stop=(dc == 1),
                        )
                    if hc % 2 == 0:
                        nc.scalar.copy(o_sb[:, hc * 512:(hc + 1) * 512], ps)
                    else:
                        nc.vector.tensor_copy(
                            o_sb[:, hc * 512:(hc + 1) * 512], ps)
                nc.sync.dma_start(
                    out=out_d[r * 128:(r + 1) * 128, :], in_=o_sb)

            def v_proj_rb(ht, g, rbg):
                """v projection for one row block (ones column appended)."""
                rb = g * RPG + rbg
                psv = ps257.tile([128, D + 1], F32, tag="ps257")
                for kc in range(KC):
                    nc.tensor.matmul(
                        psv[:, 0:D],
                        lhsT=ht[kc // 4][:, kc % 4, rbg * 128:(rbg + 1) * 128],
                        rhs=wv_sb[:, kc, :],
                        start=(kc == 0), stop=(kc == KC - 1),
                    )
                nc.vector.tensor_copy(v_aug[:, rb, 0:D], psv[:, 0:D])
                nc.vector.memset(v_aug[:, rb, D:D + 1], 1.0)

            # ---------------- pipelined groups -------------------------------
            # rows are software-pipelined: row r's transposes + o_proj are
            # emitted after row r+1's scores so the PSUM-evacuation latency
            # chains hide under matmuls.
            ht = ht0
            pending = None   # (r, a_sb) awaiting finish + o_proj
            scored = None    # (r, order) awaiting attn·v
            for g in range(NG):
                kraw = rawpool.tile([128, 2, GW], BF16, tag="kraw")
                qraw = rawpool.tile([128, 2, GW], BF16, tag="qraw")
                proj_qk(wk_sb, kraw, ht)
                proj_qk(wq_sb, qraw, ht)
                # k-norm overlaps the v projection below on the PE
                norm_rope(kraw, kTf, gk_sb, eps_k, 1.0, g)
                v_proj_rb(ht, g, 0)
                # q-norm emitted here so its ACT/DVE chain hides under the
                # remaining v projection matmuls
                norm_rope(qraw, qTf, gq_sb, eps_q, SCALING * SCALING, g)
                v_proj_rb(ht, g, 1)
                v_proj_rb(ht, g, 2)
                v_proj_rb(ht, g, 3)

                # prefetch the next group's hT chunks (ht[g] is now consumed)
                if g + 1 < NG:
                    nxt = []
                    for dq in range(5):
                        t = htpool.tile([128, 4, GW], BF16, tag=f"ht{dq}")
                        nc.sync.dma_start(
                            out=t, in_=hT_d[:, g + 1, dq * 4:(dq + 1) * 4, :])
                        nxt.append(t)
                    ht = nxt

                # attention + o_proj for this group's row blocks,
                # software-pipelined two deep:
                # scores(r) | attn·v(r-1) | transpose+o_proj(r-2)
                for rbg in range(RPG):
                    r = g * RPG + rbg
                    order = attention_row(r)
                    if scored is not None:
                        sr, so = scored
                        a_sb = attnv_row(so)
                        if pending is not None:
                            pr, pa = pending
                            finish_row(pr, pa)
                            o_proj_row(pr)
                        pending = (sr, a_sb)
                    scored = (r, order)
            sr, so = scored
            a_sb = attnv_row(so)
            pr, pa = pending
            finish_row(pr, pa)
            o_proj_row(pr)
            finish_row(sr, a_sb)
            o_proj_row(sr)

    return nc


def make_in_maps(hidden_states, cos, sin, w_qkv, w_o, q_norm_w, k_norm_w):
    """Host-side sharding / relayout: one input map per core."""
    f32 = np.float32
    hT = _chunk_part(np.ascontiguousarray(hidden_states.T).astype(f32))
    # regroup to [128, NG seq-groups, KC, GW] so each group loads with a
    # few large contiguous DMAs
    hT = _bf16(np.ascontiguousarray(
        hT.reshape(128, KC, NG, GW).transpose(0, 2, 1, 3)))

    cosT = np.ascontiguousarray(cos.T).astype(f32)   # [D, S]
    sinT = np.ascontiguousarray(sin.T).astype(f32)
    cosH = _bf16(np.ascontiguousarray(cosT[:128]))   # cos duplicated halves
    sinH = _bf16(np.ascontiguousarray(sinT[:128]))

    def gain_tables(w):
        """lhsT for the column-sum matmul: 1/(1+w)^2 broadcast along cols."""
        g = 1.0 / np.square(1.0 + w.astype(f32))     # [D]
        G = np.empty((128, 2, 128), f32)
        G[:, 0, :] = g[:128, None]
        G[:, 1, :] = g[128:, None]
        return G

    Gq = gain_tables(q_norm_w)
    Gk = gain_tables(k_norm_w)
    gq1 = 1.0 + q_norm_w.astype(f32)
    gk1 = 1.0 + k_norm_w.astype(f32)

    jj = np.arange(128)[:, None]  # key index within block (partition)
    ii = np.arange(128)[None, :]  # query index within block (free)
    bmask = np.concatenate([
        np.where(jj >= ii + 1, 1.0, 0.0),   # partial (c == r - BLK_WIN)
        np.where(jj <= ii, 1.0, 0.0),       # diagonal (c == r)
    ], axis=1).astype(f32)
    bmask = _bf16(bmask)

    in_maps = []
    for h in range(N_CORES):
        g = h // (NH // NKV)
        wq = _bf16(_chunk_part(np.ascontiguousarray(
            w_qkv[:, h * D:(h + 1) * D]).astype(f32) * gq1[None, :]))
        wk = _bf16(_chunk_part(np.ascontiguousarray(
            w_qkv[:, NH * D + g * D: NH * D + (g + 1) * D]
        ).astype(f32) * gk1[None, :]))
        wv = _bf16(_chunk_part(np.ascontiguousarray(
            w_qkv[:, (NH + NKV) * D + g * D: (NH + NKV) * D + (g + 1) * D]
        ).astype(f32)))
        wo = _bf16(_chunk_part(np.ascontiguousarray(
            w_o[h * D:(h + 1) * D, :]).astype(f32)))
        in_maps.append({
            "hT": hT, "wq": wq, "wk": wk, "wv": wv,
            "cosH": cosH, "sinH": sinH, "Gq": Gq, "Gk": Gk,
            "wo": wo, "bmask": bmask,
        })
    return in_maps


_NC_CACHE = None


def _get_nc():
    global _NC_CACHE
    if _NC_CACHE is None:
        _NC_CACHE = split_multiwaits(build_nc())
    return _NC_CACHE


def run(inputs, trace=False, **kw):
    """Returns (full_output, BassKernelResults)."""
    nc = _get_nc()
    in_maps = make_in_maps(**inputs)
    res = run_bass_kernel_spmd(
        nc, in_maps, core_ids=list(range(N_CORES)), trace=trace, **kw
    )
    parts = [res.results[i]["out"].astype(np.float32) for i in range(N_CORES)]
    out = np.sum(np.stack(parts, axis=0), axis=0, dtype=np.float32)
    return out, res


def kernel(**inputs) -> np.ndarray:
    out, _ = run(inputs, trace=False)
    return out
